# revision 4
# baseline (speedup 1.0000x reference)
"""Trainium2 Bass kernel for nn_CustomAttn: fused QKV + RoPE + causal SDPA + out-proj.

Sharding: tensor-parallel over heads (16 heads / 8 cores = 2 heads/core).
Each core computes QKV for its 2 heads (d-major layouts), RoPE, causal
flash-style attention (scores kept transposed [k, q] so softmax-normalization
and the PV matmul need no per-block transposes), producing attn^T feature-major
[128, tokens]. An AllGather over the partition axis assembles the full
attn^T [1024, tokens]; each core then computes its 128-row slice of
y^T = w_out @ attn^T.  Host assembles y from the 8 row-slices.

All matmuls run in float32r (TF32-like: ~1.5e-4 rel err, 4x faster than fp32).
"""
import sys

if "/opt/trn_rl_repo" not in sys.path:
    sys.path.insert(0, "/opt/trn_rl_repo")

import numpy as np

import concourse.bass as bass
import concourse.tile as tile
from concourse import bacc, mybir
from concourse.bass_utils import run_bass_kernel_spmd
from concourse.masks import make_identity

F32 = mybir.dt.float32
F32R = mybir.dt.float32r
EXP = mybir.ActivationFunctionType.Exp

B, S, D, H, HD = 2, 2048, 1024, 16, 64
NCORE = 8
HPC = H // NCORE  # 2 heads per core
TOK = B * S  # 4096 flattened tokens
ST = 512  # s-tile / q-tile width
NST = TOK // ST  # 8
NQT = S // ST  # 4 q-tiles per batch
KB = 128  # k-block
NKB_B = S // KB  # 16 k-blocks per batch
DCH = D // 128  # 8 contraction chunks
SCALE = 1.0 / np.sqrt(HD)
ROPE_BASE = 10000.0

_CACHE: dict = {}


def _build_program():
    nc = bacc.Bacc("TRN2", target_bir_lowering=False, debug=False, num_devices=NCORE)

    # ---- DRAM I/O ----
    xT_d = nc.dram_tensor("xT", [D, TOK], F32R, kind="ExternalInput").ap()
    wq_d = nc.dram_tensor("wq", [D, 128], F32R, kind="ExternalInput").ap()
    wk_d = nc.dram_tensor("wk", [D, 128], F32R, kind="ExternalInput").ap()
    wv_d = nc.dram_tensor("wv", [D, 128], F32R, kind="ExternalInput").ap()
    wo_d = nc.dram_tensor("wo", [D, 128], F32R, kind="ExternalInput").ap()
    cos_d = nc.dram_tensor("cosT", [128, S], F32, kind="ExternalInput").ap()
    sin_d = nc.dram_tensor("sinT", [128, S], F32, kind="ExternalInput").ap()
    mask_d = nc.dram_tensor("masks", [NQT, 128, ST], F32R, kind="ExternalInput").ap()
    yt_d = nc.dram_tensor("yt", [128, TOK], F32, kind="ExternalOutput").ap()

    with tile.TileContext(nc) as tc:
        with (
            tc.tile_pool(name="const", bufs=1) as cpool,
            tc.tile_pool(name="persist", bufs=1) as ppool,
            tc.tile_pool(name="xt", bufs=2) as xpool,
            tc.tile_pool(name="rope", bufs=2) as rpool,
            tc.tile_pool(name="e", bufs=3) as epool,
            tc.tile_pool(name="at", bufs=2) as apool,
            tc.tile_pool(name="rz", bufs=2) as zpool,
            tc.tile_pool(name="agin", bufs=1) as gpool,
            tc.tile_pool(name="yt", bufs=2) as ypool,
            tc.tile_pool(name="pqkv", bufs=3, space="PSUM") as pqkv,
            tc.tile_pool(name="pscr", bufs=3, space="PSUM") as pscr,
            tc.tile_pool(name="po", bufs=2, space="PSUM") as po,
            tc.tile_pool(name="dram", bufs=1, space="DRAM") as dpool,
        ):
            # ---- constants / weights ----
            wq_sb = cpool.tile([128, DCH, 128], F32R)
            nc.sync.dma_start(wq_sb[:], wq_d.rearrange("(a p) m -> p a m", p=128))
            wk_sb = cpool.tile([128, DCH, 128], F32R)
            nc.sync.dma_start(wk_sb[:], wk_d.rearrange("(a p) m -> p a m", p=128))
            wv_sb = cpool.tile([128, DCH, 128], F32R)
            nc.sync.dma_start(wv_sb[:], wv_d.rearrange("(a p) m -> p a m", p=128))
            wo_sb = cpool.tile([128, DCH, 128], F32R)
            nc.sync.dma_start(wo_sb[:], wo_d.rearrange("(a p) m -> p a m", p=128))
            cos_sb = cpool.tile([128, S], F32)
            nc.sync.dma_start(cos_sb[:], cos_d)
            sin_sb = cpool.tile([128, S], F32)
            nc.sync.dma_start(sin_sb[:], sin_d)
            mask_sb = cpool.tile([128, NQT, ST], F32R)
            nc.sync.dma_start(mask_sb[:], mask_d.rearrange("j p q -> p j q"))
            id_sb = cpool.tile([128, 128], F32)
            make_identity(nc, id_sb[:])
            onesf = cpool.tile([128, 1], F32)
            nc.vector.memset(onesf[:], 1.0)
            onesf64 = cpool.tile([1, 64], F32)
            nc.vector.memset(onesf64[:], 1.0)
            ones_r = cpool.tile([1, 64], F32R)
            nc.vector.tensor_copy(ones_r[:], onesf64[:])

            # ---- persistent activations ----
            qt_all = ppool.tile([128, TOK], F32R)  # RoPE'd Q^T (2 heads stacked)
            kt_all = ppool.tile([128, TOK], F32R)  # RoPE'd K^T
            # token-major V per 128-token block, per-head [64 V | 1 ones] slots
            v_all = ppool.tile([128, 2 * NKB_B, 2 * (HD + 1)], F32R)

            def rope(dst, src_ps, s0):
                """dst[128,ST] (f32r) = src*cos + rotate_half(src)*sin_signed."""
                stg = rpool.tile([128, ST], F32, tag="stg")
                nc.vector.tensor_copy(stg[:], src_ps[:])
                rot = rpool.tile([128, ST], F32, tag="rot")
                for h0 in (0, 64):
                    nc.vector.tensor_copy(
                        rot[h0 : h0 + 32, :], stg[h0 + 32 : h0 + 64, :]
                    )
                    nc.vector.tensor_copy(
                        rot[h0 + 32 : h0 + 64, :], stg[h0 : h0 + 32, :]
                    )
                t1 = rpool.tile([128, ST], F32, tag="t1")
                nc.vector.tensor_mul(t1[:], stg[:], cos_sb[:, s0 : s0 + ST])
                nc.vector.tensor_mul(rot[:], rot[:], sin_sb[:, s0 : s0 + ST])
                nc.vector.tensor_add(dst, t1[:], rot[:])

            # ---- phase 1: QKV projection + RoPE + V transpose ----
            for st in range(NST):
                s0 = (st % NQT) * ST  # within-batch position (cos/sin index)
                tok0 = st * ST
                xt_sb = xpool.tile([128, DCH, ST], F32R, tag="xt")
                nc.sync.dma_start(
                    xt_sb[:],
                    xT_d.rearrange("(a p) m -> p a m", p=128)[
                        :, :, tok0 : tok0 + ST
                    ],
                )
                for part, w_sb in (("q", wq_sb), ("k", wk_sb), ("v", wv_sb)):
                    acc = pqkv.tile([128, ST], F32, tag="qkv", name=f"ps_{part}{st}")
                    for dk in range(DCH):
                        nc.tensor.matmul(
                            acc[:],
                            w_sb[:, dk, :],
                            xt_sb[:, dk, :],
                            start=(dk == 0),
                            stop=(dk == DCH - 1),
                        )
                    if part == "q":
                        rope(qt_all[:, tok0 : tok0 + ST], acc, s0)
                    elif part == "k":
                        rope(kt_all[:, tok0 : tok0 + ST], acc, s0)
                    else:
                        vs = rpool.tile([128, ST], F32, tag="stg", name=f"vs{st}")
                        nc.vector.tensor_copy(vs[:], acc[:])
                        for jj in range(ST // KB):
                            slot = st * (ST // KB) + jj
                            tr = pscr.tile(
                                [128, 128], F32, tag="scr", name=f"tr{slot}"
                            )
                            nc.tensor.transpose(
                                tr[:], vs[:, jj * 128 : (jj + 1) * 128], id_sb[:]
                            )
                            for h in range(HPC):
                                c0 = h * (HD + 1)
                                nc.vector.tensor_copy(
                                    v_all[:, slot, c0 : c0 + HD],
                                    tr[:, h * HD : (h + 1) * HD],
                                )
                                nc.vector.tensor_copy(
                                    v_all[:, slot, c0 + HD : c0 + HD + 1], onesf[:]
                                )

            # ---- phase 2: causal attention per (batch, q-tile) ----
            ag_in = []
            for b in range(B):
                t = dpool.tile([128, S], F32R, name=f"ag_in{b}")
                ag_in.append(t)
            for b in range(B):
                for qt in range(NQT):
                    q0 = b * S + qt * ST
                    nkb = (qt + 1) * (ST // KB)
                    o_ps = [
                        po.tile([HD + 1, ST], F32, tag="o", name=f"o{b}_{qt}_{h}")
                        for h in range(HPC)
                    ]
                    for kbi in range(nkb):
                        slot = b * NKB_B + kbi
                        k0 = b * S + kbi * KB
                        for h in range(HPC):
                            stp = pscr.tile(
                                [128, ST], F32, tag="scr", name=f"st{b}_{qt}_{kbi}_{h}"
                            )
                            nc.tensor.matmul(
                                stp[:],
                                kt_all[h * HD : (h + 1) * HD, k0 : k0 + KB],
                                qt_all[h * HD : (h + 1) * HD, q0 : q0 + ST],
                                start=True,
                                stop=True,
                            )
                            e_sb = epool.tile(
                                [128, ST], F32R, tag="e", name=f"e{b}_{qt}_{kbi}_{h}"
                            )
                            nc.scalar.activation(e_sb[:], stp[:], EXP, scale=SCALE)
                            j = kbi - qt * (ST // KB)
                            if j >= 0:
                                nc.vector.tensor_mul(
                                    e_sb[:], e_sb[:], mask_sb[:, j, :]
                                )
                            c0 = h * (HD + 1)
                            nc.tensor.matmul(
                                o_ps[h][:],
                                v_all[:, slot, c0 : c0 + HD + 1],
                                e_sb[:],
                                start=(kbi == 0),
                                stop=(kbi == nkb - 1),
                            )
                    for h in range(HPC):
                        rz = zpool.tile([1, ST], F32, tag="rz")
                        nc.vector.reciprocal(rz[:], o_ps[h][HD : HD + 1, :])
                        rzr = zpool.tile([1, ST], F32R, tag="rzr")
                        nc.vector.tensor_copy(rzr[:], rz[:])
                        bc = pscr.tile(
                            [HD, ST], F32, tag="scr", name=f"bc{b}_{qt}_{h}"
                        )
                        nc.tensor.matmul(
                            bc[:], ones_r[:], rzr[:], start=True, stop=True
                        )
                        bc_sb = zpool.tile([HD, ST], F32, tag="bcs")
                        nc.vector.tensor_copy(bc_sb[:], bc[:])
                        at_sb = apool.tile([HD, ST], F32R, tag="at")
                        nc.vector.tensor_mul(at_sb[:], o_ps[h][0:HD, :], bc_sb[:])
                        nc.sync.dma_start(
                            ag_in[b][h * HD : (h + 1) * HD, qt * ST : (qt + 1) * ST],
                            at_sb[:],
                        )

            # ---- phase 3: AllGather + out-projection ----
            ag_out = []
            for b in range(B):
                t = dpool.tile(
                    [D, S], F32R, addr_space="Shared", name=f"ag_out{b}"
                )
                ag_out.append(t)
                nc.gpsimd.collective_compute(
                    "AllGather",
                    mybir.AluOpType.bypass,
                    replica_groups=[list(range(NCORE))],
                    ins=[ag_in[b].opt()],
                    outs=[t.opt()],
                )
            for tt in range(NST):
                b = tt // NQT
                c0 = (tt % NQT) * ST
                ag_sb = gpool.tile([128, DCH, ST], F32R, tag="ag")
                nc.sync.dma_start(
                    ag_sb[:],
                    ag_out[b].rearrange("(a p) m -> p a m", p=128)[
                        :, :, c0 : c0 + ST
                    ],
                )
                yt_ps = pqkv.tile([128, ST], F32, tag="qkv", name=f"yt{tt}")
                for fk in range(DCH):
                    nc.tensor.matmul(
                        yt_ps[:],
                        wo_sb[:, fk, :],
                        ag_sb[:, fk, :],
                        start=(fk == 0),
                        stop=(fk == DCH - 1),
                    )
                yt_sb = ypool.tile([128, ST], F32, tag="yt")
                nc.vector.tensor_copy(yt_sb[:], yt_ps[:])
                nc.sync.dma_start(yt_d[:, tt * ST : (tt + 1) * ST], yt_sb[:])

    nc.compile()
    return nc


def _host_tables():
    inv_freq = 1.0 / (ROPE_BASE ** (np.arange(0, HD, 2, dtype=np.float32) / HD))
    t = np.arange(S, dtype=np.float32)
    freqs = np.outer(t, inv_freq)  # [S, 32]
    emb = np.concatenate([freqs, freqs], axis=-1)  # [S, 64]
    cos = np.cos(emb).astype(np.float32)
    sin = np.sin(emb).astype(np.float32)
    sinS = np.concatenate([-sin[:, : HD // 2], sin[:, HD // 2 :]], axis=1)
    cosT2 = np.ascontiguousarray(np.concatenate([cos.T, cos.T], axis=0))  # [128,S]
    sinT2 = np.ascontiguousarray(np.concatenate([sinS.T, sinS.T], axis=0))
    kk = np.arange(128)[:, None]
    qq = np.arange(ST)[None, :]
    masks = np.zeros((NQT, 128, ST), dtype=np.float32)
    for j in range(NQT):
        masks[j] = (j * 128 + kk <= qq).astype(np.float32)
    return cosT2, sinT2, masks


def _get_nc():
    if "nc" not in _CACHE:
        _CACHE["nc"] = _build_program()
        _CACHE["tables"] = _host_tables()
    return _CACHE["nc"]


def _make_in_maps(x, w_in, w_out):
    cosT2, sinT2, masks = _CACHE["tables"]
    xT = np.ascontiguousarray(x.reshape(TOK, D).T)  # [D, TOK]
    in_maps = []
    for c in range(NCORE):
        r = slice(c * 128, (c + 1) * 128)
        in_maps.append(
            {
                "xT": xT,
                "wq": np.ascontiguousarray(w_in[0 * D :][r.start : r.stop].T),
                "wk": np.ascontiguousarray(w_in[1 * D :][r.start : r.stop].T),
                "wv": np.ascontiguousarray(w_in[2 * D :][r.start : r.stop].T),
                "wo": np.ascontiguousarray(w_out[r, :].T),
                "cosT": cosT2,
                "sinT": sinT2,
                "masks": masks,
            }
        )
    return in_maps


def kernel(x: np.ndarray, w_in: np.ndarray, w_out: np.ndarray) -> np.ndarray:
    x = np.asarray(x, dtype=np.float32)
    w_in = np.asarray(w_in, dtype=np.float32)
    w_out = np.asarray(w_out, dtype=np.float32)

    nc = _get_nc()
    in_maps = _make_in_maps(x, w_in, w_out)
    res = run_bass_kernel_spmd(nc, in_maps, core_ids=list(range(NCORE)))
    yT = np.concatenate([res.results[c]["yt"] for c in range(NCORE)], axis=0)
    return np.ascontiguousarray(yT.T).reshape(B, S, D)
